# revision 75
# baseline (speedup 1.0000x reference)
"""DeformableConv2D (B=8, C=F=256, H=W=64, K=3x3) on 8 Trainium2 NeuronCores.

Sharding: data-parallel over batch - each of the 8 cores processes one sample.

Per-core pipeline (v12):
  1. offset/mask 3x3 SAME convs as shifted bf16 matmuls (f32 PSUM); xpad1 is
     loaded in three column windows so the conv starts ~3us in.
  2. Four 16-row stripes: as conv rows land, pixel-partition PE transposes,
     a pymod-based floor/frac pipeline, gather-index DMAs (+1-hop 16->128
     replicate), corner-product planes, plane-row transposes, the stripe's
     dma_gathers AND (4 gathers behind) the per-unit compute are all emitted,
     so every engine's queue priority-interleaves with the gather stream.
  3. The gather stream (36.5 x 6827ns on the GPSIMD queue) is the hard
     floor; all other engines are budgeted under 6827ns/unit:
       DVE: one merged corner-multiply + one merged yc-add (~6.6us)
       Act: 4 plane-broadcast PSUM->bf16 copies (~4.2us) + stripe copies
       PE:  4 identity-column broadcasts + lag-2 GEMM (~5.2us)
  4. The final (chunk 3, k=8) unit is split into two 512-px halves to halve
     the post-stream drain. Output staged bf16; host converts to f32.

kernel(**inputs) takes the FULL batch and returns the FULL [8,256,64,64] f32
output.
"""

import dataclasses
from collections import deque
from contextlib import ExitStack

import numpy as np

import concourse.bass as bass
import concourse.bacc as bacc
import concourse.tile as tile
from concourse import mybir
from concourse.bass_utils import run_bass_kernel_spmd

H = W = 64
HW = H * W
C = 256
F = 256
K = 9
OC = 41  # conv out rows: 0-8 dy, 9-17 dx, 32-40 mask
PAD = 8
HP = H + 2 * PAD  # 80
WP = W + 2 * PAD  # 80
NROW = HP * WP  # 6400
H1 = H + 2  # 66
W1 = W + 2
HW1 = H1 * W1  # 4356

FP32 = mybir.dt.float32
I32 = mybir.dt.int32
BF16 = mybir.dt.bfloat16
I16 = mybir.dt.int16
AX = mybir.AluOpType
AF = mybir.ActivationFunctionType

CHUNK = 1024
NCHUNK = HW // CHUNK  # 4
Q = 4 * K  # 36 planes
NCORES = 8
LAG = 4  # units trail the gather stream by this many gathers
GLAG = 2  # GEMM trails its unit by this many units on the PE queue


def host_inputs(x, w_offset, w_mask, w_deform):
    """Per-sample layout prep. x: [C,H,W] float32 one sample."""
    import ml_dtypes

    ins = {}
    xp1 = np.zeros((C, H1, W1), ml_dtypes.bfloat16)
    xp1[:, 1:-1, 1:-1] = x
    ins["xpad1"] = np.ascontiguousarray(xp1.reshape(C, HW1))

    # xg2 row (y, x) = [xpad[y, x, :], xpad[y+1, x, :]]  (bf16)
    xp2 = np.zeros((HP + 1, WP, C), ml_dtypes.bfloat16)
    xp2[PAD : PAD + H, PAD : PAD + W, :] = np.transpose(x, (1, 2, 0)).astype(
        ml_dtypes.bfloat16
    )
    xg2 = np.concatenate([xp2[:-1], xp2[1:]], axis=2)  # [HP, WP, 2C]
    ins["xg2"] = np.ascontiguousarray(xg2.reshape(NROW, 2 * C))

    # conv weights, out-channel order [dy(9) | dx(9) | pad | mask(9) at 32]
    wt = np.zeros((3, 3, C, OC), np.float32)
    wo = np.transpose(w_offset, (2, 3, 1, 0))  # [3,3,C,18]
    wt[:, :, :, 0:9] = wo[:, :, :, 0::2]  # dy_k = offset channel 2k
    wt[:, :, :, 9:18] = wo[:, :, :, 1::2]  # dx_k = offset channel 2k+1
    wt[:, :, :, 32:41] = np.transpose(w_mask, (2, 3, 1, 0))
    ins["wconv"] = np.ascontiguousarray(
        wt.reshape(K, 2, 128, OC), dtype=ml_dtypes.bfloat16
    )

    wd = np.transpose(w_deform.reshape(F, C, K), (2, 1, 0))  # [k, c, f]
    ins["wdef"] = np.ascontiguousarray(
        wd.reshape(K, 2, 128, F).astype(ml_dtypes.bfloat16)
    )

    p = np.arange(HW)
    hh = (p // W).astype(np.float32)
    ww = (p % W).astype(np.float32)
    ky = np.repeat(np.arange(3) - 1, 3).astype(np.float32)
    kx = np.tile(np.arange(3) - 1, 3).astype(np.float32)
    basey = (hh[:, None] + ky[None, :]).reshape(32, 128, K).transpose(1, 0, 2)
    basex = (ww[:, None] + kx[None, :]).reshape(32, 128, K).transpose(1, 0, 2)
    ins["basey"] = np.ascontiguousarray(basey, dtype=np.float32)
    ins["basex"] = np.ascontiguousarray(basex, dtype=np.float32)
    ins["ident"] = np.eye(128, dtype=np.float32)
    ins["identb"] = np.eye(128, dtype=ml_dtypes.bfloat16)
    sel = np.zeros((Q, Q, 128), ml_dtypes.bfloat16)
    for q in range(Q):
        sel[q, q, :] = 1.0
    ins["sel"] = sel.reshape(Q, Q * 128)
    return ins


def declare_inputs(nc):
    t = {}
    t["xpad1"] = nc.dram_tensor("xpad1", [C, HW1], BF16, kind="ExternalInput")
    t["xg2"] = nc.dram_tensor("xg2", [NROW, 2 * C], BF16, kind="ExternalInput")
    t["wconv"] = nc.dram_tensor("wconv", [K, 2, 128, OC], BF16, kind="ExternalInput")
    t["wdef"] = nc.dram_tensor("wdef", [K, 2, 128, F], BF16, kind="ExternalInput")
    t["basey"] = nc.dram_tensor("basey", [128, 32, K], FP32, kind="ExternalInput")
    t["basex"] = nc.dram_tensor("basex", [128, 32, K], FP32, kind="ExternalInput")
    t["ident"] = nc.dram_tensor("ident", [128, 128], FP32, kind="ExternalInput")
    t["identb"] = nc.dram_tensor("identb", [128, 128], BF16, kind="ExternalInput")
    t["sel"] = nc.dram_tensor("sel", [Q, Q * 128], BF16, kind="ExternalInput")
    # columns in wrapped-j order: j = 16*(8a + t) + b <-> pixel 128t + 16a + b
    t["out"] = nc.dram_tensor("out", [F, HW], BF16, kind="ExternalOutput")
    return t


def build(nc, tc, ctx: ExitStack, t):
    keep = ctx.enter_context(tc.tile_pool(name="keep", bufs=1))

    ident = keep.tile([128, 128], FP32)
    identb = keep.tile([128, 128], BF16)
    wdef_sb = keep.tile([128, K * 2 * F], BF16)
    widx = keep.tile([128, K, HW // 16], I16)
    plrow = keep.tile([Q, HW], BF16)  # wrapped-j order plane rows
    sel = keep.tile([Q, Q * 128], BF16)
    scr = keep.tile([128, 2], FP32)

    def load_aux():
        nc.scalar.dma_start(ident[:], t["ident"].ap())
        nc.scalar.dma_start(identb[:], t["identb"].ap())
        nc.scalar.dma_start(sel[:], t["sel"].ap())
        # warm the Act sigmoid table now so the 1.3us LoadActFuncSet is off
        # the stripe-0 critical path
        nc.vector.memset(scr[:], 0.0)
        nc.scalar.activation(scr[:, 0:1], scr[:, 0:1], AF.Sigmoid)

    def load_wdef():
        # needed only by the first GEMM, well into the gather stream
        nc.sync.dma_start(
            wdef_sb[:].rearrange("p (k c f) -> p k c f", k=K, c=2),
            t["wdef"].ap().rearrange("k c p f -> p k c f"),
        )

    # gather pool up front (gathers are emitted inside the prologue); the
    # main-loop compute pools are created after the prologue closes so they
    # reuse its SBUF/PSUM space.
    gp = ctx.enter_context(tc.tile_pool(name="gth", bufs=4))
    pools = {}

    xg_in = dataclasses.replace(
        t["xg2"].ap(), ap=[[2 * C, NROW - 1], [1, 2 * 2 * C]]
    )  # overlapping row pairs, elem = 4 corners x 256ch
    units = []
    for c in range(NCHUNK):
        for k in range(K):
            if c == NCHUNK - 1 and k == K - 1:
                units.append((c, k, 1024 * c, 512))
                units.append((c, k, 1024 * c + 512, 512))
            else:
                units.append((c, k, 1024 * c, 1024))
    stripe_units = [range(0, 9), range(9, 18), range(18, 27), range(27, 37)]
    gtiles = {}
    wdef_v = wdef_sb[:].rearrange("p (k c f) -> p k c f", k=K, c=2)

    def bsel(q):
        # [Q, 128] stationary view of identb with a 0-stride free dim:
        # stat[i, p] = identb[i, q] = (i == q) -- broadcasts plrow row q to
        # all 128 PSUM partitions.
        v = identb[:]
        return dataclasses.replace(
            v, ap=[[v.ap[0][0], Q], [0, 128]], offset=v.offset + q
        )

    ps_out = {}
    out_cols = {c: 0 for c in range(NCHUNK)}
    stiles = {}
    gemm_q = deque()

    def emit_gemm(v):
        ch, k, col0, width = units[v]
        base = col0 - 1024 * ch
        s01 = stiles.pop(v)
        if ch not in ps_out:
            ps_out[ch] = [
                pools['gps'].tile([128, CHUNK], FP32, tag=f"ops{m}", name=f"ops{ch}_{m}")
                for m in range(2)
            ]
        first = k == 0
        last = k == K - 1
        for m in range(2):
            for si in range(2):
                for ci in range(2):
                    for n0 in range(0, width, 512):
                        nc.tensor.matmul(
                            ps_out[ch][m][:, base + n0 : base + n0 + 512],
                            wdef_v[:, k, ci, bass.ts(m, 128)],
                            s01[:, si, ci, n0 : n0 + 512],
                            start=(first and si == 0 and ci == 0),
                            stop=(last and si == 1 and ci == 1),
                        )
        if last:
            for m in range(2):
                ot = pools['op'].tile(
                    [128, CHUNK], BF16, tag="ot", name=f"ot{ch}_{m}_{base}"
                )
                nc.scalar.copy(ot[:, :width], ps_out[ch][m][:, base : base + width])
                nc.sync.dma_start(
                    t["out"].ap()[bass.ts(m, 128), col0 : col0 + width],
                    ot[:, :width],
                )
            out_cols[ch] += width
            if out_cols[ch] >= 1024:
                ps_out.pop(ch)

    def emit_unit(v):
        ch, k, col0, width = units[v]
        g = gtiles.pop(v)
        # 4 plane-row broadcasts (identity-column stationary) -> bf16 prep
        # quadrants via Act copies; the yc-add later reuses prep's storage.
        a = pools['ap'].tile([128, 2, 2, 2, CHUNK], BF16, tag="am", name=f"am{v}")
        for xc in range(2):
            prep = pools['pr'].tile(
                [128, 2, CHUNK], BF16, tag="prep", name=f"pr{v}_{xc}"
            )
            for yc in range(2):
                q = (2 * xc + yc) * K + k
                brd = pools['bps'].tile(
                    [128, CHUNK], FP32, tag="brd", name=f"brd{v}_{xc}{yc}"
                )
                for n0 in range(0, width, 512):
                    nc.tensor.matmul(
                        brd[:, n0 : n0 + 512],
                        sel[:, 128 * q : 128 * q + 128],
                        plrow[:, col0 + n0 : col0 + n0 + 512],
                        start=True,
                        stop=True,
                    )
                nc.scalar.copy(prep[:, yc, :width], brd[:, :width])
            pr_ap = prep[:]
            pr_b = dataclasses.replace(
                pr_ap,
                ap=[list(pr_ap.ap[0]), [CHUNK, 2], [0, 2], [1, width]],
            )
            nc.vector.tensor_tensor(
                a[:, xc, :, :, :width],
                g[:, 4 * xc : 4 * xc + 4, :width].rearrange(
                    "p (y c) j -> p y c j", y=2
                ),
                pr_b,
                AX.mult,
            )
        s01 = pools['sp'].tile([128, 2, 2, CHUNK], BF16, tag="s01", name=f"s01_{v}")
        nc.vector.tensor_add(
            s01[:, :, :, :width], a[:, :, 0, :, :width], a[:, :, 1, :, :width]
        )
        stiles[v] = s01
        gemm_q.append(v)
        if len(gemm_q) > GLAG:
            emit_gemm(gemm_q.popleft())

    def emit_gather(u):
        ci_, k, col0, width = units[u]
        g = gp.tile(
            [128, 8, width],
            BF16,
            tag=f"g{width}",
            name=f"g{u}",
            bufs=4 if width == 1024 else 2,
        )
        nc.gpsimd.dma_gather(
            g[:],
            xg_in,
            widx[:, k, col0 // 16 : (col0 + width) // 16],
            num_idxs=width,
            num_idxs_reg=width,
            elem_size=2 * 2 * C,
            elem_step=2 * C,
            transpose=True,
            single_packet=False,
        )
        gtiles[u] = g

    # ================= prologue =================
    with tc.tile_pool(name="prol", bufs=1) as prol, tc.tile_pool(
        name="stgp", bufs=4
    ) as stgp, tc.tile_pool(
        name="prps", bufs=2, space="PSUM"
    ) as prps, tc.tile_pool(name="trps", bufs=3, space="PSUM") as trps:
        wconv_sb = prol.tile([128, K * 2 * OC], BF16, tag="wconv")
        nc.sync.dma_start(
            wconv_sb[:].rearrange("p (k c o) -> p k c o", k=K, c=2),
            t["wconv"].ap().rearrange("k c p o -> p k c o"),
        )
        # xpad1 loaded in three column windows per ci half so the first conv
        # blocks start ~3us in. Window w covers xp1 columns [lo, hi).
        XWIN = [(-68, 1100), (857, 2950), (2705, 4424)]
        xp1w = [
            [
                prol.tile(
                    [128, hi - lo], BF16, tag=f"xp1_{w}_{i}", name=f"xp1_{w}_{i}"
                )
                for i in range(2)
            ]
            for w, (lo, hi) in enumerate(XWIN)
        ]
        for w, (lo, hi) in enumerate(XWIN):
            d0 = max(lo, 0)
            d1 = min(hi, HW1)
            for i in range(2):
                tl = xp1w[w][i]
                if lo < 0:
                    nc.vector.memset(tl[:, 0:-lo], 0.0)
                if hi > HW1:
                    nc.vector.memset(tl[:, HW1 - lo :], 0.0)
                eng = (nc.sync, nc.scalar)[i]
                eng.dma_start(
                    tl[:, d0 - lo : d1 - lo],
                    t["xpad1"].ap()[bass.ts(i, 128), d0:d1],
                )

        def xw(ci, j0):
            w = 0 if j0 < 924 else (1 if j0 < 2772 else 2)
            return xp1w[w][ci], XWIN[w][0]

        load_aux()

        # conv into two bf16 half tiles; A: rows 0..41; B: rows 42..65
        JSPLIT = 42 * W1  # 2772 (6 blocks of 7 rows)
        convA = prol.tile([OC, JSPLIT], BF16, tag="convA")
        convB = prol.tile([OC, HW1 - JSPLIT], BF16, tag="convB")
        NCONV = 7 * W1  # 462 (7 rows, 1 PSUM bank)
        wviews = wconv_sb[:].rearrange("p (k c o) -> p k c o", k=K, c=2)
        pixT = prol.tile([128, 32, OC], FP32, tag="pixT")

        def conv_row(h):  # [OC, W1] view of conv output row h
            if (h + 1) * W1 <= JSPLIT:
                return convA[:, h * W1 : (h + 1) * W1]
            return convB[:, h * W1 - JSPLIT : (h + 1) * W1 - JSPLIT]

        def emit_transpose(tcol):
            h0 = 2 * tcol
            stage = stgp.tile([OC, 128], FP32, tag="tr_stage", name=f"st{tcol}")
            ceng = nc.vector.tensor_copy if tcol % 2 else nc.scalar.copy
            for r in range(2):
                ceng(
                    stage[:, 64 * r : 64 * r + 64],
                    conv_row(h0 + 1 + r)[:, 1 : 1 + W],
                )
            ps = trps.tile([128, OC], FP32, tag="tr_ps", name=f"trp{tcol}")
            nc.tensor.transpose(ps[:], stage[:], ident[:OC, :OC])
            peng = nc.vector.tensor_copy if tcol % 2 else nc.scalar.copy
            peng(pixT[:, tcol, :], ps[:])

        def emit_conv_block(j0):
            n = min(NCONV, HW1 - j0)
            ps = prps.tile([OC, NCONV], FP32, tag="conv_ps", name=f"cv{j0}")
            first = True
            for ci in range(2):
                xt, lo = xw(ci, j0)
                for k in range(K):
                    off = (k // 3 - 1) * W1 + (k % 3 - 1)
                    nc.tensor.matmul(
                        ps[:, :n],
                        wviews[:, k, ci, :],
                        xt[:, j0 + off - lo : j0 + off - lo + n],
                        start=first,
                        stop=(ci == 1 and k == K - 1),
                    )
                    first = False
            ceng = nc.scalar.copy if (j0 // NCONV) % 2 else nc.vector.tensor_copy
            if j0 < JSPLIT:
                ceng(convA[:, j0 : j0 + n], ps[:, :n])
            else:
                ceng(convB[:, j0 - JSPLIT : j0 - JSPLIT + n], ps[:, :n])

        # ---- per-stripe coefficient/index pipeline ----
        def pt2(tag):
            return prol.tile([128, 32, 2 * K], FP32, tag=tag, name=tag)

        typ = pt2("typ")
        fyx = pt2("fyx")
        wyx = pt2("wyx")
        cr = pt2("cr")
        iy = prol.tile([128, 32, 2 * K], I32, tag="iy")
        mwy0 = prol.tile([128, 32, K], BF16, tag="mwy0", name="mwy0")
        mwy1 = prol.tile([128, 32, K], BF16, tag="mwy1", name="mwy1")
        base2 = prol.tile([128, 32, 2 * K], FP32, tag="base2")
        nc.sync.dma_start(base2[:, :, 0:9], t["basey"].ap())
        nc.sync.dma_start(base2[:, :, 9:18], t["basex"].ap())
        load_wdef()
        CONST = PAD * WP + PAD
        idxt = prol.tile([128, K, 32], FP32, tag="idxt")
        idxi = prol.tile([128, K, 32], I16, tag="idxi")
        coefq = prol.tile([128, Q, 32], BF16, tag="coefq")

        def emit_stripe(s):
            ts = slice(8 * s, 8 * s + 8)
            nc.scalar.activation(
                pixT[:, ts, 32:41], pixT[:, ts, 32:41], AF.Sigmoid
            )
            # fpos = floor(dv + base), robust to trunc-or-round casts
            nc.vector.tensor_add(typ[:, ts], pixT[:, ts, 0:18], base2[:, ts])
            nc.vector.tensor_copy(iy[:, ts], typ[:, ts])
            nc.vector.tensor_copy(fyx[:, ts], iy[:, ts])
            nc.vector.tensor_tensor(cr[:, ts], fyx[:, ts], typ[:, ts], AX.is_gt)
            nc.vector.tensor_sub(fyx[:, ts], fyx[:, ts], cr[:, ts])
            nc.vector.tensor_sub(wyx[:, ts], typ[:, ts], fyx[:, ts])
            fy = fyx[:, ts, 0:9]
            fx = fyx[:, ts, 9:18]
            wy = wyx[:, ts, 0:9]
            wx = wyx[:, ts, 9:18]
            mv = pixT[:, ts, 32:41]
            # gather indices (exact integers in f32, so the i16 cast is safe)
            iv = idxt[:, :, ts].rearrange("p q t -> p t q")
            nc.vector.scalar_tensor_tensor(iv, fy, float(WP), fx, AX.mult, AX.add)
            nc.vector.tensor_scalar(
                idxt[:, :, ts],
                idxt[:, :, ts],
                float(CONST),
                float(NROW - 2),
                AX.add,
                AX.min,
            )
            nc.vector.tensor_scalar(
                idxt[:, :, ts], idxt[:, :, ts], 0.0, None, AX.max
            )
            nc.vector.tensor_copy(idxi[:, :, ts], idxt[:, :, ts])
            # wrap + replicate: 8 partition-block writes, then 7 INDEPENDENT
            # copies of partitions [0:16) (one dependency hop)
            for a in range(8):
                eng = (nc.sync, nc.scalar)[a % 2]
                eng.dma_start(
                    widx[0:16, :, 64 * s + 8 * a : 64 * s + 8 * a + 8],
                    idxi[16 * a : 16 * a + 16, :, ts],
                )
            for st in range(3):
                w = 16 << st
                eng = (nc.sync, nc.scalar)[s % 2]
                eng.dma_start(
                    widx[w : 2 * w, :, 64 * s : 64 * s + 64],
                    widx[0:w, :, 64 * s : 64 * s + 64],
                )
            # corner-product planes
            nc.vector.tensor_mul(mwy1[:, ts], mv, wy)
            nc.vector.tensor_sub(mwy0[:, ts], mv, mwy1[:, ts])
            cv = coefq[:, :, ts].rearrange("p q t -> p t q")
            nc.vector.tensor_mul(cv[:, :, 18:27], mwy0[:, ts], wx)
            nc.vector.tensor_sub(cv[:, :, 0:9], mwy0[:, ts], cv[:, :, 18:27])
            nc.vector.tensor_mul(cv[:, :, 27:36], mwy1[:, ts], wx)
            nc.vector.tensor_sub(cv[:, :, 9:18], mwy1[:, ts], cv[:, :, 27:36])
            for tcol in range(8 * s, 8 * s + 8):
                tm = tcol - 8 * s
                stage2 = stgp.tile([128, Q], FP32, tag="tr2_stage", name=f"s2{tcol}")
                ceng = nc.vector.tensor_copy if tcol % 2 else nc.scalar.copy
                ceng(stage2[:], coefq[:, :, tcol])
                ceng2 = ceng
                ps = trps.tile([Q, 128], FP32, tag="tr2_ps", name=f"t2p{tcol}")
                nc.tensor.transpose(ps[:], stage2[:], ident[:, :])
                dstr = plrow[:, 0:128]
                dstr = dataclasses.replace(
                    dstr,
                    ap=[list(dstr.ap[0]), [128, 8], [1, 16]],
                    offset=dstr.offset + 1024 * s + 16 * tm,
                )
                ceng2(dstr, ps[:].rearrange("q (a b) -> q a b", a=8))

        # interleave pixT transposes (and each stripe's index/plane chain +
        # gathers + trailing unit compute) into the conv as rows land
        pend = list(range(32))

        def drain_ready(rows_done):
            while pend and 2 * pend[0] + 2 < rows_done:
                tcol = pend.pop(0)
                emit_transpose(tcol)
                if tcol % 8 == 7:
                    s = tcol // 8
                    emit_stripe(s)
                    for u in stripe_units[s]:
                        emit_gather(u)

        for j0 in range(0, HW1, NCONV):
            emit_conv_block(j0)
            drain_ready(min(j0 + NCONV, HW1) // W1)
        for tcol in list(pend):
            pend.pop(0)
            emit_transpose(tcol)
            if tcol % 8 == 7:
                s = tcol // 8
                emit_stripe(s)
                for u in stripe_units[s]:
                    emit_gather(u)

    # ================= main loop =================
    pools['ap'] = ctx.enter_context(tc.tile_pool(name="amul", bufs=3))
    pools['pr'] = ctx.enter_context(tc.tile_pool(name="prep", bufs=3))
    pools['sp'] = ctx.enter_context(tc.tile_pool(name="sums", bufs=3))
    pools['op'] = ctx.enter_context(tc.tile_pool(name="outp", bufs=2))
    pools['gps'] = ctx.enter_context(tc.tile_pool(name="gemm_ps", bufs=1, space="PSUM"))
    pools['bps'] = ctx.enter_context(tc.tile_pool(name="brd_ps", bufs=2, space="PSUM"))
    for v in range(len(units)):
        emit_unit(v)
    while gemm_q:
        emit_gemm(gemm_q.popleft())


_CACHE = {}


def _get_nc():
    if "nc" not in _CACHE:
        nc = bacc.Bacc("TRN2", target_bir_lowering=False, num_devices=NCORES)
        t = declare_inputs(nc)
        with tile.TileContext(nc) as tc:
            with ExitStack() as ctx:
                build(nc, tc, ctx, t)
        nc.finalize()
        _CACHE["nc"] = nc
    return _CACHE["nc"]


def kernel(x, w_offset, w_mask, w_deform):
    """Full-batch deformable conv. x: [8,256,64,64] f32 -> [8,256,64,64] f32."""
    x = np.asarray(x, dtype=np.float32)
    w_offset = np.asarray(w_offset, dtype=np.float32)
    w_mask = np.asarray(w_mask, dtype=np.float32)
    w_deform = np.asarray(w_deform, dtype=np.float32)
    B = x.shape[0]
    assert B == NCORES
    nc = _get_nc()
    in_maps = [host_inputs(x[b], w_offset, w_mask, w_deform) for b in range(B)]
    res = run_bass_kernel_spmd(nc, in_maps, list(range(NCORES)))
    out = np.empty((B, F, H, W), np.float32)
    for b in range(B):
        o = np.asarray(res.results[b]["out"], dtype=np.float32)
        o = o.reshape(F, 4, 8, 8, 16)  # (s, a, tm, b)
        out[b] = o.transpose(0, 1, 3, 2, 4).reshape(F, H, W)
    return out
